# revision 1
# baseline (speedup 1.0000x reference)
"""GAT-style bipartite graph attention layer (nn_BiGraphContrastLayer) on 8 trn2 cores.

Strategy (dst-sharded SPMD, one shared program):
  - Every core computes zel = x @ [W | W@Al | W@Ar] for all N nodes (replicated;
    bf16 matmul, fp32 accum) and writes a per-node row table
    zel_tab[n] = [z(512) | el(8) | er(8) | pad] (bf16, 640 elems = 1280B) to DRAM.
  - Each core owns 1250 dst nodes.  Their incoming edges (+ self loops), sorted
    by dst and grouped into 10 dst tiles of 128, are gathered per edge from
    zel_tab via SWDGE dma_gather (src row: 1280B; dst el/er tail: 256B).
  - v = exp(leaky_relu(el_src + er_dst)) per edge/head; messages msg = v * z_src
    (DVE, per-head broadcast); per-dst-tile segment sums via one-hot selection
    matmuls on the PE accumulating in PSUM: out_tile = SelT.T @ msg and
    s_tile = SelT.T @ v.  Final: out/s + bias.
  No inter-core communication; host concatenates the 8 dst slices.
"""
import os

import numpy as np
import ml_dtypes

import concourse.bacc as bacc
import concourse.bass as bass
import concourse.mybir as mybir
import concourse.tile as tile

BF = ml_dtypes.bfloat16
F32 = np.float32

NS, ND, E, DIN, H, DH = 10000, 10000, 320000, 512, 8, 64
NEG = 0.2
NCORES = 8
DPC = ND // NCORES          # 1250 dst nodes per core
N = NS + ND
NPAD = 20480                # node count padded to 160 tiles of 128
ROW = 640                   # zel row elems: z(512) | el(8) | er(8) | pad(112)
NTILES = (DPC + 127) // 128  # 10 dst tiles per core
PANEL = 2048                # phase-1 node panel (16 subtiles of 128)


# ----------------------------------------------------------------- host prep
def _wrap_idx(idx):
    """dma_gather index layout: idx i -> [i % 16, i // 16], replicated 8x."""
    k = len(idx)
    w = np.zeros((16, k // 16), np.int16)
    w[np.arange(k) % 16, np.arange(k) // 16] = idx
    return np.tile(w, (8, 1))


def _host_prep(x_src, x_dst, edge_src, edge_dst, W, attn_l, attn_r, bias):
    x = np.concatenate([x_src, x_dst], 0).astype(F32)       # [N, 512]
    xT = np.zeros((DIN, NPAD), BF)
    xT[:, :N] = x.T
    Al = np.zeros((DIN, H), F32)
    Ar = np.zeros((DIN, H), F32)
    for h in range(H):
        Al[h * DH:(h + 1) * DH, h] = attn_l[h]
        Ar[h * DH:(h + 1) * DH, h] = attn_r[h]
    Wext = np.concatenate([W, W @ Al, W @ Ar], 1).astype(BF)  # [512, 528]
    bias_rep = np.tile(bias[None, :].astype(F32), (128, 1))   # [128, 512]

    # per-(core, dst tile) edge lists, sorted by local dst
    edge_src = edge_src.astype(np.int64)
    edge_dst = edge_dst.astype(np.int64)
    tlists = [[None] * NTILES for _ in range(NCORES)]
    kmax = 0
    for c in range(NCORES):
        d0 = c * DPC
        m = (edge_dst >= d0) & (edge_dst < d0 + DPC)
        es = np.concatenate([edge_src[m],
                             NS + d0 + np.arange(DPC, dtype=np.int64)])
        ed = np.concatenate([edge_dst[m] - d0, np.arange(DPC, dtype=np.int64)])
        order = np.argsort(ed, kind="stable")
        es, ed = es[order], ed[order]
        for t in range(NTILES):
            sel = (ed >= t * 128) & (ed < (t + 1) * 128)
            tlists[c][t] = (es[sel], ed[sel] - t * 128)
            kmax = max(kmax, int(sel.sum()))
    k_tile = ((kmax + 127) // 128) * 128
    nch = k_tile // 128

    per_core = []
    for c in range(NCORES):
        zidx = np.zeros((128, NTILES * k_tile // 16), np.int16)
        selT = np.zeros((128, NTILES * nch * 128), BF)
        selD = np.zeros((128, NTILES * nch * 128), BF)
        for t in range(NTILES):
            es, edl = tlists[c][t]
            k = len(es)
            src = np.zeros(k_tile, np.int64)
            src[:k] = es
            s16 = slice(t * k_tile // 16, (t + 1) * k_tile // 16)
            zidx[:, s16] = _wrap_idx(src)
            dstl = np.full(k_tile, -1, np.int64)
            dstl[:k] = edl
            for ch in range(nch):
                dl = dstl[ch * 128:(ch + 1) * 128]
                sm = np.zeros((128, 128), F32)
                valid = dl >= 0
                sm[np.arange(128)[valid], dl[valid]] = 1.0
                j = (t * nch + ch) * 128
                selT[:, j:j + 128] = sm.astype(BF)
                selD[:, j:j + 128] = sm.T.astype(BF)
        eridx = _wrap_idx(NS + c * DPC + np.arange(NTILES * 128, dtype=np.int64))
        per_core.append(dict(selT=selT, selD=selD, zidx=zidx, eridx=eridx))

    shared = dict(xT=xT, Wext=Wext, bias_rep=bias_rep)
    return shared, per_core, k_tile, nch


# ------------------------------------------------------------- bass program
def _build_nc(k_tile, nch):
    nc = bacc.Bacc("TRN2", target_bir_lowering=False, debug=False)
    dt = mybir.dt

    xT_d = nc.dram_tensor("xT", [DIN, NPAD], dt.bfloat16, kind="ExternalInput")
    W_d = nc.dram_tensor("Wext", [DIN, 528], dt.bfloat16, kind="ExternalInput")
    bias_d = nc.dram_tensor("bias_rep", [128, 512], dt.float32, kind="ExternalInput")
    selT_d = nc.dram_tensor("selT", [128, NTILES * nch * 128], dt.bfloat16,
                            kind="ExternalInput")
    selD_d = nc.dram_tensor("selD", [128, NTILES * nch * 128], dt.bfloat16,
                            kind="ExternalInput")
    zidx_d = nc.dram_tensor("zidx", [128, NTILES * k_tile // 16], dt.int16,
                            kind="ExternalInput")
    eridx_d = nc.dram_tensor("eridx", [128, NTILES * 128 // 16], dt.int16,
                             kind="ExternalInput")
    out_d = nc.dram_tensor("out", [NTILES * 128, 512], dt.float32,
                           kind="ExternalOutput")
    zel_d = nc.dram_tensor("zel_tab", [NPAD, ROW], dt.bfloat16)

    with tile.TileContext(nc) as tc:
        # ---- constants resident in SBUF
        with tc.tile_pool(name="const", bufs=1) as cpool:
            wsb = cpool.tile([128, 4 * 528], dt.bfloat16)
            for k in range(4):
                nc.sync.dma_start(wsb[:, k * 528:(k + 1) * 528],
                                  W_d[k * 128:(k + 1) * 128, :])
            bias_sb = cpool.tile([128, 512], dt.float32)
            nc.sync.dma_start(bias_sb[:], bias_d[:])
            zidx_sb = cpool.tile([128, NTILES * k_tile // 16], dt.int16)
            nc.sync.dma_start(zidx_sb[:], zidx_d[:])
            eridx_sb = cpool.tile([128, NTILES * 128 // 16], dt.int16)
            nc.sync.dma_start(eridx_sb[:], eridx_d[:])

            # ---- phase 1: zel_tab = [x@W | x@Wl | x@Wr] for all nodes
            with (
                tc.tile_pool(name="xp", bufs=2) as xpool,
                tc.tile_pool(name="zel", bufs=3) as zpool,
                tc.tile_pool(name="p1", bufs=2, space="PSUM") as p1pool,
                tc.tile_pool(name="p1b", bufs=2, space="PSUM") as p1bpool,
            ):
                for p in range(NPAD // PANEL):
                    xp = xpool.tile([128, 4 * PANEL], dt.bfloat16)
                    for k in range(4):
                        nc.sync.dma_start(
                            xp[:, k * PANEL:(k + 1) * PANEL],
                            xT_d[k * 128:(k + 1) * 128,
                                 p * PANEL:(p + 1) * PANEL])
                    for m in range(PANEL // 128):
                        zps = p1pool.tile([128, 512], dt.float32, space="PSUM")
                        lps = p1bpool.tile([128, 16], dt.float32, space="PSUM")
                        for k in range(4):
                            lhsT = xp[:, k * PANEL + m * 128:
                                      k * PANEL + (m + 1) * 128]
                            nc.tensor.matmul(zps[:], lhsT,
                                             wsb[:, k * 528:k * 528 + 512],
                                             start=(k == 0), stop=(k == 3))
                            nc.tensor.matmul(lps[:], lhsT,
                                             wsb[:, k * 528 + 512:(k + 1) * 528],
                                             start=(k == 0), stop=(k == 3))
                        zel_sb = zpool.tile([128, ROW], dt.bfloat16)
                        nc.vector.tensor_copy(zel_sb[:, 0:512], zps[:])
                        nc.vector.tensor_copy(zel_sb[:, 512:528], lps[:])
                        nc.gpsimd.memset(zel_sb[:, 528:ROW], 0)
                        row0 = (p * (PANEL // 128) + m) * 128
                        nc.sync.dma_start(zel_d[row0:row0 + 128, :], zel_sb[:])

            # all phase-1 zel_tab writes must land before gathers read it
            tc.strict_bb_all_engine_barrier()

            # ---- phase 2: per dst tile gather + attention + aggregation
            with (
                tc.tile_pool(name="zg", bufs=3) as zgpool,
                tc.tile_pool(name="era", bufs=1) as erapool,
                tc.tile_pool(name="sel", bufs=3) as selpool,
                tc.tile_pool(name="sc", bufs=3) as scpool,
                tc.tile_pool(name="eo", bufs=2) as eopool,
                tc.tile_pool(name="p2", bufs=3, space="PSUM") as p2pool,
                tc.tile_pool(name="p2b", bufs=3, space="PSUM") as p2bpool,
                tc.tile_pool(name="p2c", bufs=2, space="PSUM") as p2cpool,
            ):
                # er (and el) of this core's dst nodes: one small gather
                era = erapool.tile([128, NTILES, 128], dt.bfloat16)
                nc.gpsimd.dma_gather(
                    era[:], zel_d[:, 512:640], eridx_sb[:],
                    num_idxs=NTILES * 128, num_idxs_reg=NTILES * 128,
                    elem_size=128, elem_step=ROW, single_packet=False)

                for t in range(NTILES):
                    zg = zgpool.tile([128, nch, ROW], dt.bfloat16)
                    sel = selpool.tile([128, nch * 128], dt.bfloat16)
                    nc.sync.dma_start(
                        sel[:], selT_d[:, t * nch * 128:(t + 1) * nch * 128])
                    seld = selpool.tile([128, nch * 128], dt.bfloat16,
                                        tag="seld")
                    nc.sync.dma_start(
                        seld[:], selD_d[:, t * nch * 128:(t + 1) * nch * 128])

                    # er_dst broadcast to edges via Sel matmuls — all chunks
                    # packed into one PSUM bank.
                    lt = scpool.tile([128, nch, 8], dt.float32, tag="lt")
                    pe_er = p2cpool.tile([128, nch, 8], dt.float32,
                                         space="PSUM")
                    for ch in range(nch):
                        nc.tensor.matmul(pe_er[:, ch, :],
                                         seld[:, ch * 128:(ch + 1) * 128],
                                         era[:, t, 8:16],
                                         start=True, stop=True,
                                         skip_group_check=True)
                    i16 = slice(t * k_tile // 16, (t + 1) * k_tile // 16)
                    nc.gpsimd.dma_gather(
                        zg[:], zel_d[:], zidx_sb[:, i16],
                        num_idxs=k_tile, num_idxs_reg=k_tile, elem_size=ROW,
                        single_packet=False)
                    nc.vector.tensor_tensor(
                        lt[:], zg[:, :, 512:520], pe_er[:],
                        op=mybir.AluOpType.add)
                    nc.vector.scalar_tensor_tensor(
                        lt[:], lt[:], NEG, lt[:],
                        op0=mybir.AluOpType.mult, op1=mybir.AluOpType.max)
                    vt = scpool.tile([128, nch, 8], dt.float32, tag="vt")
                    nc.scalar.activation(vt[:], lt[:],
                                         mybir.ActivationFunctionType.Exp)
                    vb = scpool.tile([128, nch, 8], dt.bfloat16, tag="vb")
                    nc.vector.tensor_copy(vb[:], vt[:])

                    # msg = v * z  (in place over the z part of zg)
                    z4 = zg[:, :, 0:512].rearrange("p c (h d) -> p c h d", d=DH)
                    nc.vector.tensor_tensor(
                        z4, z4, vb[:].to_broadcast([128, nch, 8, DH]),
                        op=mybir.AluOpType.mult)

                    # segment sums on the PE
                    po = p2pool.tile([128, 512], dt.float32, space="PSUM")
                    ps = p2bpool.tile([128, 8], dt.float32, space="PSUM")
                    for ch in range(nch):
                        sl = sel[:, ch * 128:(ch + 1) * 128]
                        nc.tensor.matmul(po[:], sl, zg[:, ch, 0:512],
                                         start=(ch == 0), stop=(ch == nch - 1))
                        nc.tensor.matmul(ps[:], sl, vb[:, ch, :],
                                         start=(ch == 0), stop=(ch == nch - 1))

                    # out = po / s + bias  (eps keeps pad rows finite: 1/eps * 0 = 0)
                    ssb = scpool.tile([128, 8], dt.float32, tag="ssb")
                    nc.vector.tensor_scalar_add(ssb[:], ps[:], 1e-30)
                    nc.vector.reciprocal(ssb[:], ssb[:])
                    osb = eopool.tile([128, 512], dt.float32)
                    o4 = osb[:].rearrange("p (h d) -> p h d", d=DH)
                    nc.vector.tensor_tensor(
                        o4, po[:].rearrange("p (h d) -> p h d", d=DH),
                        ssb[:].to_broadcast([128, 8, DH]),
                        op=mybir.AluOpType.mult)
                    nc.vector.tensor_tensor(osb[:], osb[:], bias_sb[:],
                                            op=mybir.AluOpType.add)
                    nc.sync.dma_start(out_d[t * 128:(t + 1) * 128, :], osb[:])
    nc.compile()
    return nc


# ------------------------------------------------------------------- driver
def kernel(x_src, x_dst, edge_src, edge_dst, W, attn_l, attn_r, bias):
    shared, per_core, k_tile, nch = _host_prep(
        np.asarray(x_src), np.asarray(x_dst), np.asarray(edge_src),
        np.asarray(edge_dst), np.asarray(W), np.asarray(attn_l),
        np.asarray(attn_r), np.asarray(bias))

    nc = _build_nc(k_tile, nch)

    in_maps = []
    for c in range(NCORES):
        in_maps.append({"xT": shared["xT"], "Wext": shared["Wext"],
                        "bias_rep": shared["bias_rep"],
                        "selT": per_core[c]["selT"],
                        "selD": per_core[c]["selD"],
                        "zidx": per_core[c]["zidx"],
                        "eridx": per_core[c]["eridx"]})

    if os.environ.get("KERNEL_SIM"):
        from concourse.bass_interp import CoreSim
        sim = CoreSim(nc, trace=False)
        for name, arr in in_maps[int(os.environ.get("KERNEL_SIM_CORE", "0"))].items():
            sim.tensor(name)[:] = arr
        sim.simulate()
        out = np.array(sim.tensor("out"))
        return np.concatenate([out[:DPC]] * NCORES, 0)  # core-0 slice only

    from concourse.bass_utils import run_bass_kernel_spmd
    res = run_bass_kernel_spmd(nc, in_maps, core_ids=list(range(NCORES)),
                               trace=bool(os.environ.get("KERNEL_TRACE")))
    global LAST_RESULTS
    LAST_RESULTS = res
    return np.concatenate([r["out"][:DPC] for r in res.results], 0)


LAST_RESULTS = None



# revision 10
# speedup vs baseline: 1.1510x; 1.1510x over previous
"""GAT-style bipartite graph attention layer (nn_BiGraphContrastLayer) on 8 trn2 cores.

Strategy (dst-sharded SPMD, one shared program, per-core node renumbering):
  - Each core works with a LOCAL node table of NLOC=11280 rows:
    rows [0, 1280): its own 1250 dst nodes, permuted into 10 balanced bins
    of 128 (padded with zero rows); rows [1280, 11280): all 10000 src nodes.
    The host permutes each core's xT input accordingly, so the shared
    program uses identical (static) access patterns on every core.
  - Phase 1: zel_tab[n] = [z(512) | el(8) | er(8)] bf16, z = x @ W etc., for
    the 11520-row padded local table (44% less work than the replicated-
    all-nodes variant).  PSUM->SBUF casts alternate Vector/Scalar engines.
  - Phase 2 per dst bin (10 bins): real incoming edges, sorted per bin and
    chunked by 128, are fetched from zel_tab with SWDGE dma_gather in
    prepare_only mode + trigger_dma on 2 queues, so descriptor generation
    overlaps phase 1 and transfers pipeline with compute.  Self-loops use
    static DMAs (own-dst rows are at fixed offsets) with an identity
    selection matrix.  v = exp(leaky_relu(el_src + er_dst)); one-hot
    selection matmuls (fp8 sel matrices) segment-sum messages and weights
    in PSUM.  out = po/s + bias.
  No inter-core communication; the host unpermutes the 8 dst slices.
"""
import os

import numpy as np
import ml_dtypes

import concourse.bacc as bacc
import concourse.bass as bass
import concourse.mybir as mybir
import concourse.tile as tile

BF = ml_dtypes.bfloat16
F32 = np.float32
F8 = ml_dtypes.float8_e4m3fn

NS, ND, E, DIN, H, DH = 10000, 10000, 320000, 512, 8, 64
NEG = 0.2
NCORES = 8
DPC = ND // NCORES          # 1250 dst nodes per core
NTILES = 10                 # dst bins per core (128 slots each)
NSLOT = NTILES * 128        # 1280 dst slots
NLOC = NSLOT + NS           # local node rows: own dsts | all srcs
NPAD2 = 11520               # padded to 90 tiles of 128 (6 panels of 1920)
GATHER_PREP = int(os.environ.get("KERNEL_GATHER_PREP", "0"))
GATHER_QUEUES = int(os.environ.get("KERNEL_GATHER_QUEUES", "1"))
GATHER_SP = int(os.environ.get("KERNEL_GATHER_SP", "0"))
PAD_ROW = NLOC              # all-zero row used by pad gather indices
ROW = 640                   # zel row stride in elems (1280B, 256B-aligned)
PANEL = 1920                # phase-1 node panel (15 subtiles of 128)


# ----------------------------------------------------------------- host prep
def _wrap_idx(idx):
    """dma_gather index layout: idx i -> [i % 16, i // 16], replicated 8x."""
    k = len(idx)
    w = np.zeros((16, k // 16), np.int16)
    w[np.arange(k) % 16, np.arange(k) // 16] = idx
    return np.tile(w, (8, 1))


def _host_prep(x_src, x_dst, edge_src, edge_dst, W, attn_l, attn_r, bias):
    Al = np.zeros((DIN, H), F32)
    Ar = np.zeros((DIN, H), F32)
    for h in range(H):
        Al[h * DH:(h + 1) * DH, h] = attn_l[h]
        Ar[h * DH:(h + 1) * DH, h] = attn_r[h]
    Wext = np.concatenate([W, W @ Al, W @ Ar], 1).astype(BF)  # [512, 528]
    bias_rep = np.tile(bias[None, :].astype(F32), (128, 1))   # [128, 512]
    ident = np.eye(128, dtype=F8)                             # [128, 128]

    edge_src = edge_src.astype(np.int64)
    edge_dst = edge_dst.astype(np.int64)

    # first pass: per-core balanced bin assignment -> global k_tile
    per_core_raw = []
    kmax = 0
    for c in range(NCORES):
        d0 = c * DPC
        m = (edge_dst >= d0) & (edge_dst < d0 + DPC)
        es = edge_src[m]
        ed = edge_dst[m] - d0
        deg = np.bincount(ed, minlength=DPC)
        # LPT greedy: heaviest nodes first onto least-loaded feasible bin
        order = np.argsort(-deg, kind="stable")
        bin_nodes = [[] for _ in range(NTILES)]
        bin_load = np.zeros(NTILES, np.int64)
        for d in order:
            feas = [b for b in range(NTILES) if len(bin_nodes[b]) < 128]
            b = min(feas, key=lambda b: bin_load[b])
            bin_nodes[b].append(d)
            bin_load[b] += deg[d]
        kmax = max(kmax, int(bin_load.max()))
        per_core_raw.append((es, ed, bin_nodes))
    k_tile = ((kmax + 127) // 128) * 128
    nch = k_tile // 128

    per_core = []
    for c in range(NCORES):
        es, ed, bin_nodes = per_core_raw[c]
        # slot of each local dst node
        perm = np.full(NSLOT, -1, np.int64)      # slot -> local dst id
        slot_of = np.full(DPC, -1, np.int64)     # local dst id -> slot
        for b in range(NTILES):
            for j, d in enumerate(bin_nodes[b]):
                s = b * 128 + j
                perm[s] = d
                slot_of[d] = s
        eslot = slot_of[ed]                       # slot of each edge's dst
        ebin = eslot // 128

        zidx = np.zeros((128, NTILES * k_tile // 16), np.int16)
        selT = np.zeros((128, NTILES * nch * 128), F8)
        selD = np.zeros((128, NTILES * nch * 128), F8)
        for t in range(NTILES):
            sel_e = ebin == t
            srcs = es[sel_e]
            slots = eslot[sel_e] - t * 128
            o = np.argsort(srcs, kind="stable")   # src-sorted within bin
            srcs, slots = srcs[o], slots[o]
            k = len(srcs)
            rows = np.full(k_tile, PAD_ROW, np.int64)
            rows[:k] = NSLOT + srcs
            s16 = slice(t * k_tile // 16, (t + 1) * k_tile // 16)
            zidx[:, s16] = _wrap_idx(rows)
            sl = np.full(k_tile, -1, np.int64)
            sl[:k] = slots
            for ch in range(nch):
                d = sl[ch * 128:(ch + 1) * 128]
                sm = np.zeros((128, 128), F32)
                valid = d >= 0
                sm[np.arange(128)[valid], d[valid]] = 1.0
                j = (t * nch + ch) * 128
                selT[:, j:j + 128] = sm.astype(F8)
                selD[:, j:j + 128] = sm.T.astype(F8)

        # permuted xT: cols [0,1280) own dsts (pads zero), [1280,11280) srcs
        xT = np.zeros((DIN, NPAD2), BF)
        dst_cols = x_dst[c * DPC + perm]          # perm=-1 rows overwritten
        dst_cols[perm < 0] = 0.0
        xT[:, :NSLOT] = dst_cols.T
        xT[:, NSLOT:NLOC] = x_src.T
        per_core.append(dict(xT=xT, selT=selT, selD=selD, zidx=zidx,
                             perm=perm))

    shared = dict(Wext=Wext, bias_rep=bias_rep, ident=ident)
    return shared, per_core, k_tile, nch


# ------------------------------------------------------------- bass program
def _build_nc(k_tile, nch):
    nc = bacc.Bacc("TRN2", target_bir_lowering=False, debug=False,
                   num_swdge_queues=GATHER_QUEUES)
    dt = mybir.dt

    xT_d = nc.dram_tensor("xT", [DIN, NPAD2], dt.bfloat16, kind="ExternalInput")
    W_d = nc.dram_tensor("Wext", [DIN, 528], dt.bfloat16, kind="ExternalInput")
    bias_d = nc.dram_tensor("bias_rep", [128, 512], dt.float32,
                            kind="ExternalInput")
    ident_d = nc.dram_tensor("ident", [128, 128], dt.float8e4,
                             kind="ExternalInput")
    selT_d = nc.dram_tensor("selT", [128, NTILES * nch * 128], dt.float8e4,
                            kind="ExternalInput")
    selD_d = nc.dram_tensor("selD", [128, NTILES * nch * 128], dt.float8e4,
                            kind="ExternalInput")
    zidx_d = nc.dram_tensor("zidx", [128, NTILES * k_tile // 16], dt.int16,
                            kind="ExternalInput")
    out_d = nc.dram_tensor("out", [NSLOT, 512], dt.float32,
                           kind="ExternalOutput")
    zel_d = nc.dram_tensor("zel_tab", [NPAD2, ROW], dt.bfloat16)

    gsem = [nc.alloc_semaphore(f"gsem{t}") for t in range(NTILES)]

    with tile.TileContext(nc) as tc:
        with tc.tile_pool(name="const", bufs=1) as cpool:
            wsb = cpool.tile([128, 4 * 528], dt.bfloat16)
            for k in range(4):
                nc.sync.dma_start(wsb[:, k * 528:(k + 1) * 528],
                                  W_d[k * 128:(k + 1) * 128, :])
            bias_sb = cpool.tile([128, 512], dt.float32)
            nc.sync.dma_start(bias_sb[:], bias_d[:])
            ident_sb = cpool.tile([128, 128], dt.float8e4)
            nc.sync.dma_start(ident_sb[:], ident_d[:])
            zidx_sb = cpool.tile([128, NTILES * k_tile // 16], dt.int16)
            nc.sync.dma_start(zidx_sb[:], zidx_d[:])
            era = cpool.tile([128, NTILES, 16], dt.bfloat16)
            zero_sb = cpool.tile([128, ROW - 528], dt.bfloat16)
            nc.gpsimd.memset(zero_sb[:], 0)

            # phase-2 SBUF pools opened before phase-1 pools: disjoint
            # regions, so early gather preps never alias phase-1 tiles
            p2ctx = (
                tc.tile_pool(name="zg", bufs=3),
                tc.tile_pool(name="zsf", bufs=2),
                tc.tile_pool(name="sel", bufs=2),
                tc.tile_pool(name="sc", bufs=3),
                tc.tile_pool(name="eo", bufs=2),
            )
            zgpool = p2ctx[0].__enter__()
            zsfpool = p2ctx[1].__enter__()
            selpool = p2ctx[2].__enter__()
            scpool = p2ctx[3].__enter__()
            eopool = p2ctx[4].__enter__()

            # ---- phase 1: zel_tab = [x@W | x@Wl | x@Wr] for local nodes
            with (
                tc.tile_pool(name="xp", bufs=2) as xpool,
                tc.tile_pool(name="zel", bufs=3) as zpool,
                tc.tile_pool(name="p1", bufs=3, space="PSUM") as p1pool,
                tc.tile_pool(name="p1b", bufs=3, space="PSUM") as p1bpool,
            ):
                for p in range(NPAD2 // PANEL):
                    xp = xpool.tile([128, 4 * PANEL], dt.bfloat16)
                    for k in range(4):
                        nc.sync.dma_start(
                            xp[:, k * PANEL:(k + 1) * PANEL],
                            xT_d[k * 128:(k + 1) * 128,
                                 p * PANEL:(p + 1) * PANEL])
                    for m in range(PANEL // 128):
                        gm = p * (PANEL // 128) + m
                        zps = p1pool.tile([128, 512], dt.float32, space="PSUM")
                        lps = p1bpool.tile([128, 16], dt.float32, space="PSUM")
                        for k in range(4):
                            lhsT = xp[:, k * PANEL + m * 128:
                                      k * PANEL + (m + 1) * 128]
                            nc.tensor.matmul(zps[:], lhsT,
                                             wsb[:, k * 528:k * 528 + 512],
                                             start=(k == 0), stop=(k == 3))
                            nc.tensor.matmul(lps[:], lhsT,
                                             wsb[:, k * 528 + 512:(k + 1) * 528],
                                             start=(k == 0), stop=(k == 3))
                        zel_sb = zpool.tile([128, 528], dt.bfloat16)
                        if gm % 2 == 0:
                            nc.vector.tensor_copy(zel_sb[:, 0:512], zps[:])
                            nc.vector.tensor_copy(zel_sb[:, 512:528], lps[:])
                        else:
                            nc.scalar.copy(zel_sb[:, 0:512], zps[:])
                            nc.scalar.copy(zel_sb[:, 512:528], lps[:])
                        row0 = gm * 128
                        nc.sync.dma_start(zel_d[row0:row0 + 128, 0:528],
                                          zel_sb[:])
                        nc.sync.dma_start(zel_d[row0:row0 + 128, 528:ROW],
                                          zero_sb[:])

            # el/er of own dst slots: static strided read of rows [0, 1280)
            nc.sync.dma_start(
                era[:],
                zel_d[0:NSLOT, 512:528].rearrange("(t p) j -> p t j", p=128))

            # ---- phase 2: per dst bin gather + attention + aggregation
            with (
                tc.tile_pool(name="p2", bufs=2, space="PSUM") as p2pool,
                tc.tile_pool(name="p2b", bufs=2, space="PSUM") as p2bpool,
                tc.tile_pool(name="p2c", bufs=2, space="PSUM") as p2cpool,
            ):
                for t in range(NTILES):
                    q = t % GATHER_QUEUES
                    zg = zgpool.tile([128, nch, ROW], dt.bfloat16)
                    i16 = slice(t * k_tile // 16, (t + 1) * k_tile // 16)
                    if GATHER_PREP:
                        nc.gpsimd.dma_gather(
                            zg[:], zel_d[:], zidx_sb[:, i16],
                            num_idxs=k_tile, num_idxs_reg=k_tile,
                            elem_size=ROW, single_packet=bool(GATHER_SP),
                            prepare_only=True, sem=gsem[t], queue_num=q)
                        nc.gpsimd.trigger_dma(count=None, queue_num=q)
                    else:
                        nc.gpsimd.dma_gather(
                            zg[:], zel_d[:], zidx_sb[:, i16],
                            num_idxs=k_tile, num_idxs_reg=k_tile,
                            elem_size=ROW, single_packet=bool(GATHER_SP),
                            queue_num=q)

                    zsf = zsfpool.tile([128, 512], dt.bfloat16)
                    nc.sync.dma_start(zsf[:],
                                      zel_d[t * 128:(t + 1) * 128, 0:512])
                    sel = selpool.tile([128, nch * 128], dt.float8e4)
                    nc.sync.dma_start(
                        sel[:], selT_d[:, t * nch * 128:(t + 1) * nch * 128])
                    seld = selpool.tile([128, nch * 128], dt.float8e4,
                                        tag="seld")
                    nc.sync.dma_start(
                        seld[:], selD_d[:, t * nch * 128:(t + 1) * nch * 128])

                    # er_dst broadcast to edges via SelD matmuls
                    pe_er = p2cpool.tile([128, nch, 8], dt.float32,
                                         space="PSUM")
                    for ch in range(nch):
                        nc.tensor.matmul(pe_er[:, ch, :],
                                         seld[:, ch * 128:(ch + 1) * 128],
                                         era[:, t, 8:16],
                                         start=True, stop=True,
                                         skip_group_check=True)

                    # data-landed wait for this tile's gather (prep/trigger
                    # RAW is not auto-wired to consumers; WAR/WAW are)
                    if GATHER_PREP:
                        nc.vector.wait_ge(gsem[t], 16)
                    lt = scpool.tile([128, nch, 8], dt.float32, tag="lt")
                    nc.vector.tensor_tensor(
                        lt[:], zg[:, :, 512:520], pe_er[:],
                        op=mybir.AluOpType.add)
                    nc.vector.scalar_tensor_tensor(
                        lt[:], lt[:], NEG, lt[:],
                        op0=mybir.AluOpType.mult, op1=mybir.AluOpType.max)
                    vb = scpool.tile([128, nch, 8], dt.bfloat16, tag="vb")
                    nc.scalar.activation(vb[:], lt[:],
                                         mybir.ActivationFunctionType.Exp)

                    # self loop: lt = el + er of own slot
                    lts = scpool.tile([128, 8], dt.float32, tag="lts")
                    nc.vector.tensor_tensor(
                        lts[:], era[:, t, 0:8], era[:, t, 8:16],
                        op=mybir.AluOpType.add)
                    nc.vector.scalar_tensor_tensor(
                        lts[:], lts[:], NEG, lts[:],
                        op0=mybir.AluOpType.mult, op1=mybir.AluOpType.max)
                    vbs = scpool.tile([128, 8], dt.bfloat16, tag="vbs")
                    nc.scalar.activation(vbs[:], lts[:],
                                         mybir.ActivationFunctionType.Exp)

                    # msg = v * z  (in place over the z part)
                    z4 = zg[:, :, 0:512].rearrange("p c (h d) -> p c h d",
                                                   d=DH)
                    nc.vector.tensor_tensor(
                        z4, z4, vb[:].to_broadcast([128, nch, 8, DH]),
                        op=mybir.AluOpType.mult)
                    zs4 = zsf[:].rearrange("p (h d) -> p h d", d=DH)
                    nc.vector.tensor_tensor(
                        zs4, zs4, vbs[:].to_broadcast([128, 8, DH]),
                        op=mybir.AluOpType.mult)

                    # segment sums on the PE (gathered chunks + self chunk)
                    po = p2pool.tile([128, 512], dt.float32, space="PSUM")
                    ps = p2bpool.tile([128, 8], dt.float32, space="PSUM")
                    for ch in range(nch):
                        sl = sel[:, ch * 128:(ch + 1) * 128]
                        nc.tensor.matmul(po[:], sl, zg[:, ch, 0:512],
                                         start=(ch == 0), stop=False)
                        nc.tensor.matmul(ps[:], sl, vb[:, ch, :],
                                         start=(ch == 0), stop=False)
                    nc.tensor.matmul(po[:], ident_sb[:], zsf[:],
                                     start=False, stop=True)
                    nc.tensor.matmul(ps[:], ident_sb[:], vbs[:],
                                     start=False, stop=True)

                    # out = po / s + bias (eps keeps pad slots finite)
                    ssb = scpool.tile([128, 8], dt.float32, tag="ssb")
                    nc.vector.tensor_scalar_add(ssb[:], ps[:], 1e-30)
                    nc.vector.reciprocal(ssb[:], ssb[:])
                    osb = eopool.tile([128, 512], dt.float32)
                    o4 = osb[:].rearrange("p (h d) -> p h d", d=DH)
                    nc.vector.tensor_tensor(
                        o4, po[:].rearrange("p (h d) -> p h d", d=DH),
                        ssb[:].to_broadcast([128, 8, DH]),
                        op=mybir.AluOpType.mult)
                    nc.vector.tensor_tensor(osb[:], osb[:], bias_sb[:],
                                            op=mybir.AluOpType.add)
                    nc.sync.dma_start(out_d[t * 128:(t + 1) * 128, :], osb[:])
            for p in reversed(p2ctx):
                p.__exit__(None, None, None)
    nc.compile()
    return nc


# ------------------------------------------------------------------- driver
def kernel(x_src, x_dst, edge_src, edge_dst, W, attn_l, attn_r, bias):
    shared, per_core, k_tile, nch = _host_prep(
        np.asarray(x_src), np.asarray(x_dst), np.asarray(edge_src),
        np.asarray(edge_dst), np.asarray(W), np.asarray(attn_l),
        np.asarray(attn_r), np.asarray(bias))

    nc = _build_nc(k_tile, nch)

    in_maps = []
    for c in range(NCORES):
        in_maps.append({"xT": per_core[c]["xT"], "Wext": shared["Wext"],
                        "bias_rep": shared["bias_rep"],
                        "ident": shared["ident"],
                        "selT": per_core[c]["selT"],
                        "selD": per_core[c]["selD"],
                        "zidx": per_core[c]["zidx"]})

    def unperm(out_core, c):
        full = np.zeros((DPC, 512), F32)
        perm = per_core[c]["perm"]
        valid = perm >= 0
        full[perm[valid]] = out_core[np.nonzero(valid)[0]]
        return full

    if os.environ.get("KERNEL_SIM"):
        from concourse.bass_interp import CoreSim
        sim = CoreSim(nc, trace=False)
        cid = int(os.environ.get("KERNEL_SIM_CORE", "0"))
        for name, arr in in_maps[cid].items():
            sim.tensor(name)[:] = arr
        sim.simulate()
        out = unperm(np.array(sim.tensor("out")), cid)
        return np.concatenate([out] * NCORES, 0)  # selected core's slice only

    from concourse.bass_utils import run_bass_kernel_spmd
    res = run_bass_kernel_spmd(nc, in_maps, core_ids=list(range(NCORES)),
                               trace=bool(os.environ.get("KERNEL_TRACE")))
    global LAST_RESULTS
    LAST_RESULTS = res
    return np.concatenate([unperm(r["out"], c)
                           for c, r in enumerate(res.results)], 0)


LAST_RESULTS = None


# revision 16
# speedup vs baseline: 1.2027x; 1.0449x over previous
"""GAT-style bipartite graph attention layer (nn_BiGraphContrastLayer) on 8 trn2 cores.

Strategy (dst-sharded SPMD, one shared program, per-core node renumbering):
  - Each core works with a LOCAL node table of NLOC=11280 rows:
    rows [0, 1280): its own 1250 dst nodes, permuted into 10 balanced bins
    of 128 (padded with zero rows); rows [1280, 11280): all 10000 src nodes.
    The host permutes each core's xT input accordingly, so the shared
    program uses identical (static) access patterns on every core.
  - Phase 1: zel_tab[n] = [z(512) | el(8) | er(8)] bf16, z = x @ W etc., for
    the 11520-row padded local table (44% less work than the replicated-
    all-nodes variant).  PSUM->SBUF casts alternate Vector/Scalar engines.
  - Phase 2 per dst bin (10 bins): real incoming edges, sorted per bin and
    chunked by 128, are fetched from zel_tab with SWDGE dma_gather in
    prepare_only mode + trigger_dma on 2 queues, so descriptor generation
    overlaps phase 1 and transfers pipeline with compute.  Self-loops use
    static DMAs (own-dst rows are at fixed offsets) with an identity
    selection matrix.  v = exp(leaky_relu(el_src + er_dst)); one-hot
    selection matmuls (fp8 sel matrices) segment-sum messages and weights
    in PSUM.  out = po/s + bias.
  No inter-core communication; the host unpermutes the 8 dst slices.
"""
import os

import numpy as np
import ml_dtypes

import concourse.bacc as bacc
import concourse.bass as bass
import concourse.mybir as mybir
import concourse.tile as tile
from concourse.instruction_name_ordered_set import InstructionNameOrderedSet

BF = ml_dtypes.bfloat16
F32 = np.float32
F8 = ml_dtypes.float8_e4m3fn

NS, ND, E, DIN, H, DH = 10000, 10000, 320000, 512, 8, 64
NEG = 0.2
NCORES = 8
DPC = ND // NCORES          # 1250 dst nodes per core
NTILES = 10                 # dst bins per core (128 slots each)
NSLOT = NTILES * 128        # 1280 dst slots
NLOC = NSLOT + NS           # local node rows: own dsts | all srcs
NPAD2 = 11520               # padded to 90 tiles of 128 (9 panels of 1280)
GATHER_PREP = int(os.environ.get("KERNEL_GATHER_PREP", "0"))
GATHER_QUEUES = int(os.environ.get("KERNEL_GATHER_QUEUES", "1"))
GATHER_SP = int(os.environ.get("KERNEL_GATHER_SP", "0"))
PAD_ROW = NLOC              # all-zero row used by pad gather indices
ROW = 640                   # zel row stride in elems (1280B, 256B-aligned)
PANEL = 1280                # phase-1 node panel (10 subtiles of 128)


# ----------------------------------------------------------------- host prep
def _wrap_idx(idx):
    """dma_gather index layout: idx i -> [i % 16, i // 16], replicated 8x."""
    k = len(idx)
    w = np.zeros((16, k // 16), np.int16)
    w[np.arange(k) % 16, np.arange(k) // 16] = idx
    return np.tile(w, (8, 1))


def _host_prep(x_src, x_dst, edge_src, edge_dst, W, attn_l, attn_r, bias):
    Al = np.zeros((DIN, H), F32)
    Ar = np.zeros((DIN, H), F32)
    for h in range(H):
        Al[h * DH:(h + 1) * DH, h] = attn_l[h]
        Ar[h * DH:(h + 1) * DH, h] = attn_r[h]
    Wext = np.concatenate([W, W @ Al, W @ Ar], 1).astype(BF)  # [512, 528]
    bias_rep = np.tile(bias[None, :].astype(F32), (128, 1))   # [128, 512]
    ident = np.eye(128, dtype=F8)                             # [128, 128]

    edge_src = edge_src.astype(np.int64)
    edge_dst = edge_dst.astype(np.int64)

    # first pass: per-core balanced bin assignment -> global k_tile
    per_core_raw = []
    kmax = 0
    for c in range(NCORES):
        d0 = c * DPC
        m = (edge_dst >= d0) & (edge_dst < d0 + DPC)
        es = edge_src[m]
        ed = edge_dst[m] - d0
        deg = np.bincount(ed, minlength=DPC)
        # LPT greedy: heaviest nodes first onto least-loaded feasible bin
        order = np.argsort(-deg, kind="stable")
        bin_nodes = [[] for _ in range(NTILES)]
        bin_load = np.zeros(NTILES, np.int64)
        for d in order:
            feas = [b for b in range(NTILES) if len(bin_nodes[b]) < 128]
            b = min(feas, key=lambda b: bin_load[b])
            bin_nodes[b].append(d)
            bin_load[b] += deg[d]
        kmax = max(kmax, int(bin_load.max()))
        per_core_raw.append((es, ed, bin_nodes))
    k_tile = ((kmax + 127) // 128) * 128
    nch = k_tile // 128

    per_core = []
    for c in range(NCORES):
        es, ed, bin_nodes = per_core_raw[c]
        # slot of each local dst node
        perm = np.full(NSLOT, -1, np.int64)      # slot -> local dst id
        slot_of = np.full(DPC, -1, np.int64)     # local dst id -> slot
        for b in range(NTILES):
            for j, d in enumerate(bin_nodes[b]):
                s = b * 128 + j
                perm[s] = d
                slot_of[d] = s
        eslot = slot_of[ed]                       # slot of each edge's dst
        ebin = eslot // 128

        zidx = np.zeros((128, NTILES * k_tile // 16), np.int16)
        selT = np.zeros((128, NTILES * nch * 128), F8)
        selD = np.zeros((128, NTILES * nch * 128), F8)
        for t in range(NTILES):
            sel_e = ebin == t
            srcs = es[sel_e]
            slots = eslot[sel_e] - t * 128
            o = np.argsort(srcs, kind="stable")   # src-sorted within bin
            srcs, slots = srcs[o], slots[o]
            k = len(srcs)
            rows = np.full(k_tile, PAD_ROW, np.int64)
            rows[:k] = NSLOT + srcs
            s16 = slice(t * k_tile // 16, (t + 1) * k_tile // 16)
            zidx[:, s16] = _wrap_idx(rows)
            sl = np.full(k_tile, -1, np.int64)
            sl[:k] = slots
            for ch in range(nch):
                d = sl[ch * 128:(ch + 1) * 128]
                sm = np.zeros((128, 128), F32)
                valid = d >= 0
                sm[np.arange(128)[valid], d[valid]] = 1.0
                j = (t * nch + ch) * 128
                selT[:, j:j + 128] = sm.astype(F8)
                selD[:, j:j + 128] = sm.T.astype(F8)

        # permuted xT: cols [0,1280) own dsts (pads zero), [1280,11280) srcs
        xT = np.zeros((DIN, NPAD2), BF)
        dst_cols = x_dst[c * DPC + perm]          # perm=-1 rows overwritten
        dst_cols[perm < 0] = 0.0
        xT[:, :NSLOT] = dst_cols.T
        xT[:, NSLOT:NLOC] = x_src.T
        per_core.append(dict(xT=xT, selT=selT, selD=selD, zidx=zidx,
                             perm=perm))

    shared = dict(Wext=Wext, bias_rep=bias_rep, ident=ident)
    return shared, per_core, k_tile, nch


# ------------------------------------------------------------- bass program
def _build_nc(k_tile, nch):
    nc = bacc.Bacc("TRN2", target_bir_lowering=False, debug=False,
                   num_swdge_queues=GATHER_QUEUES)
    dt = mybir.dt

    xT_d = nc.dram_tensor("xT", [DIN, NPAD2], dt.bfloat16, kind="ExternalInput")
    W_d = nc.dram_tensor("Wext", [DIN, 528], dt.bfloat16, kind="ExternalInput")
    bias_d = nc.dram_tensor("bias_rep", [128, 512], dt.float32,
                            kind="ExternalInput")
    ident_d = nc.dram_tensor("ident", [128, 128], dt.float8e4,
                             kind="ExternalInput")
    selT_d = nc.dram_tensor("selT", [128, NTILES * nch * 128], dt.float8e4,
                            kind="ExternalInput")
    selD_d = nc.dram_tensor("selD", [128, NTILES * nch * 128], dt.float8e4,
                            kind="ExternalInput")
    zidx_d = nc.dram_tensor("zidx", [128, NTILES * k_tile // 16], dt.int16,
                            kind="ExternalInput")
    out_d = nc.dram_tensor("out", [NSLOT, 512], dt.float32,
                           kind="ExternalOutput")
    zel_d = nc.dram_tensor("zel_tab", [NPAD2, ROW], dt.bfloat16)

    gsem = [nc.alloc_semaphore(f"gsem{t}") for t in range(NTILES)]

    with tile.TileContext(nc) as tc:
        with tc.tile_pool(name="const", bufs=1) as cpool:
            wsb = cpool.tile([128, 4 * 528], dt.bfloat16)
            for k in range(4):
                nc.sync.dma_start(wsb[:, k * 528:(k + 1) * 528],
                                  W_d[k * 128:(k + 1) * 128, :])
            bias_sb = cpool.tile([128, 512], dt.float32)
            nc.sync.dma_start(bias_sb[:], bias_d[:])
            ident_sb = cpool.tile([128, 128], dt.float8e4)
            nc.sync.dma_start(ident_sb[:], ident_d[:])
            zidx_sb = cpool.tile([128, NTILES * k_tile // 16], dt.int16)
            nc.sync.dma_start(zidx_sb[:], zidx_d[:])
            era = cpool.tile([128, NTILES, 16], dt.bfloat16)
            zero_sb = cpool.tile([128, 5 * ROW], dt.bfloat16)
            nc.gpsimd.memset(zero_sb[:], 0)
            # pre-zero the whole zel table (pads stay zero; data rows are
            # overwritten by phase 1) so early gather preps see finite data
            for g in range(NPAD2 // 640):
                nc.sync.dma_start(
                    zel_d[g * 640:(g + 1) * 640, :]
                    .rearrange("(g p) j -> p g j", p=128),
                    zero_sb[:].rearrange("p (g j) -> p g j", j=ROW))

            # phase-2 SBUF pools opened before phase-1 pools: disjoint
            # regions, so early gather preps never alias phase-1 tiles
            p2ctx = (
                tc.tile_pool(name="zg", bufs=3),
                tc.tile_pool(name="zsf", bufs=2),
                tc.tile_pool(name="sel", bufs=2),
                tc.tile_pool(name="sc", bufs=3),
                tc.tile_pool(name="eo", bufs=2),
            )
            zgpool = p2ctx[0].__enter__()
            zsfpool = p2ctx[1].__enter__()
            selpool = p2ctx[2].__enter__()
            scpool = p2ctx[3].__enter__()
            eopool = p2ctx[4].__enter__()

            zel_writers = []

            def emit_prep(t):
                q = t % GATHER_QUEUES
                zg = zgpool.tile([128, nch, ROW], dt.bfloat16)
                i16 = slice(t * k_tile // 16, (t + 1) * k_tile // 16)
                prep = nc.gpsimd.dma_gather(
                    zg[:], zel_d[:], zidx_sb[:, i16],
                    num_idxs=k_tile, num_idxs_reg=k_tile,
                    elem_size=ROW, single_packet=bool(GATHER_SP),
                    prepare_only=True, sem=gsem[t], queue_num=q).ins
                # strip zel RAW deps off the prep (descgen reads only idxs);
                # they are re-attached to the trigger below
                keep = [d for d in prep.sync_dependency_names()
                        if d not in zel_writers]
                prep.set_sync_dependencies(InstructionNameOrderedSet(keep))
                return zg, prep

            preps = {}
            if GATHER_PREP:
                for t in range(min(3, NTILES)):
                    preps[t] = emit_prep(t)

            # ---- phase 1: zel_tab = [x@W | x@Wl | x@Wr] for local nodes
            with (
                tc.tile_pool(name="xp", bufs=2) as xpool,
                tc.tile_pool(name="zel", bufs=3) as zpool,
                tc.tile_pool(name="p1", bufs=3, space="PSUM") as p1pool,
                tc.tile_pool(name="p1b", bufs=3, space="PSUM") as p1bpool,
            ):
                for p in range(NPAD2 // PANEL):
                    xp = xpool.tile([128, 4 * PANEL], dt.bfloat16)
                    for k in range(4):
                        nc.sync.dma_start(
                            xp[:, k * PANEL:(k + 1) * PANEL],
                            xT_d[k * 128:(k + 1) * 128,
                                 p * PANEL:(p + 1) * PANEL])
                    for m in range(PANEL // 128):
                        gm = p * (PANEL // 128) + m
                        zps = p1pool.tile([128, 512], dt.float32, space="PSUM")
                        lps = p1bpool.tile([128, 16], dt.float32, space="PSUM")
                        for k in range(4):
                            lhsT = xp[:, k * PANEL + m * 128:
                                      k * PANEL + (m + 1) * 128]
                            nc.tensor.matmul(zps[:], lhsT,
                                             wsb[:, k * 528:k * 528 + 512],
                                             start=(k == 0), stop=(k == 3))
                            nc.tensor.matmul(lps[:], lhsT,
                                             wsb[:, k * 528 + 512:(k + 1) * 528],
                                             start=(k == 0), stop=(k == 3))
                        zel_sb = zpool.tile([128, 528], dt.bfloat16)
                        if gm % 2 == 0:
                            nc.vector.tensor_copy(zel_sb[:, 0:512], zps[:])
                            vcast = nc.vector.tensor_copy(
                                zel_sb[:, 512:528], lps[:])
                            last_vec_name = vcast.ins.name
                        else:
                            nc.scalar.copy(zel_sb[:, 0:512], zps[:])
                            nc.scalar.copy(zel_sb[:, 512:528], lps[:])
                        row0 = gm * 128
                        w = nc.sync.dma_start(zel_d[row0:row0 + 128, 0:528],
                                              zel_sb[:])
                        zel_writers.append(w.ins.name)

            # el/er of own dst slots: static strided read of rows [0, 1280)
            nc.sync.dma_start(
                era[:],
                zel_d[0:NSLOT, 512:528].rearrange("(t p) j -> p t j", p=128))

            # ---- phase 2: per dst bin gather + attention + aggregation
            with (
                tc.tile_pool(name="p2", bufs=2, space="PSUM") as p2pool,
                tc.tile_pool(name="p2b", bufs=2, space="PSUM") as p2bpool,
                tc.tile_pool(name="p2c", bufs=2, space="PSUM") as p2cpool,
            ):
                for t in range(NTILES):
                    q = t % GATHER_QUEUES
                    i16 = slice(t * k_tile // 16, (t + 1) * k_tile // 16)
                    if GATHER_PREP:
                        zg, _prep = preps[t]
                        trig = nc.gpsimd.trigger_dma(
                            count=1, queue_num=q).ins
                        trig.set_sync_dependencies(InstructionNameOrderedSet(
                            list(trig.sync_dependency_names()) + zel_writers))
                        if t + 3 < NTILES:
                            preps[t + 3] = emit_prep(t + 3)
                    else:
                        zg = zgpool.tile([128, nch, ROW], dt.bfloat16)
                        nc.gpsimd.dma_gather(
                            zg[:], zel_d[:], zidx_sb[:, i16],
                            num_idxs=k_tile, num_idxs_reg=k_tile,
                            elem_size=ROW, single_packet=bool(GATHER_SP),
                            queue_num=q)

                    zsf = zsfpool.tile([128, 512], dt.bfloat16)
                    nc.sync.dma_start(zsf[:],
                                      zel_d[t * 128:(t + 1) * 128, 0:512])
                    sel = selpool.tile([128, nch * 128], dt.float8e4)
                    nc.sync.dma_start(
                        sel[:], selT_d[:, t * nch * 128:(t + 1) * nch * 128])
                    seld = selpool.tile([128, nch * 128], dt.float8e4,
                                        tag="seld")
                    nc.sync.dma_start(
                        seld[:], selD_d[:, t * nch * 128:(t + 1) * nch * 128])

                    # er_dst broadcast to edges via SelD matmuls
                    pe_er = p2cpool.tile([128, nch, 8], dt.float32,
                                         space="PSUM")
                    for ch in range(nch):
                        nc.tensor.matmul(pe_er[:, ch, :],
                                         seld[:, ch * 128:(ch + 1) * 128],
                                         era[:, t, 8:16],
                                         start=True, stop=True,
                                         skip_group_check=True)

                    # data-landed wait for this tile's gather (prep/trigger
                    # RAW is not auto-wired to consumers; WAR/WAW are).
                    # The wait rides the first consumer; an anchor dep keeps
                    # it after prior Vector work (else the engine would park
                    # at the head of its queue and deadlock phase 1).
                    lt = scpool.tile([128, nch, 8], dt.float32, tag="lt")
                    lt_add = nc.vector.tensor_tensor(
                        lt[:], zg[:, :, 512:520], pe_er[:],
                        op=mybir.AluOpType.add)
                    if GATHER_PREP:
                        lt_add._wait_ge(gsem[t], 16)
                        lt_add.ins.set_sync_dependencies(
                            InstructionNameOrderedSet(
                                list(lt_add.ins.sync_dependency_names())
                                + [last_vec_name]))
                    nc.vector.scalar_tensor_tensor(
                        lt[:], lt[:], NEG, lt[:],
                        op0=mybir.AluOpType.mult, op1=mybir.AluOpType.max)
                    vb = scpool.tile([128, nch, 8], dt.bfloat16, tag="vb")
                    nc.scalar.activation(vb[:], lt[:],
                                         mybir.ActivationFunctionType.Exp)

                    # self loop: lt = el + er of own slot
                    lts = scpool.tile([128, 8], dt.float32, tag="lts")
                    nc.vector.tensor_tensor(
                        lts[:], era[:, t, 0:8], era[:, t, 8:16],
                        op=mybir.AluOpType.add)
                    nc.vector.scalar_tensor_tensor(
                        lts[:], lts[:], NEG, lts[:],
                        op0=mybir.AluOpType.mult, op1=mybir.AluOpType.max)
                    vbs = scpool.tile([128, 8], dt.bfloat16, tag="vbs")
                    nc.scalar.activation(vbs[:], lts[:],
                                         mybir.ActivationFunctionType.Exp)

                    # msg = v * z  (in place over the z part)
                    z4 = zg[:, :, 0:512].rearrange("p c (h d) -> p c h d",
                                                   d=DH)
                    nc.vector.tensor_tensor(
                        z4, z4, vb[:].to_broadcast([128, nch, 8, DH]),
                        op=mybir.AluOpType.mult)
                    zs4 = zsf[:].rearrange("p (h d) -> p h d", d=DH)
                    nc.vector.tensor_tensor(
                        zs4, zs4, vbs[:].to_broadcast([128, 8, DH]),
                        op=mybir.AluOpType.mult)

                    # segment sums on the PE (gathered chunks + self chunk)
                    po = p2pool.tile([128, 512], dt.float32, space="PSUM")
                    ps = p2bpool.tile([128, 8], dt.float32, space="PSUM")
                    for ch in range(nch):
                        sl = sel[:, ch * 128:(ch + 1) * 128]
                        nc.tensor.matmul(po[:], sl, zg[:, ch, 0:512],
                                         start=(ch == 0), stop=False)
                        nc.tensor.matmul(ps[:], sl, vb[:, ch, :],
                                         start=(ch == 0), stop=False)
                    nc.tensor.matmul(po[:], ident_sb[:], zsf[:],
                                     start=False, stop=True)
                    nc.tensor.matmul(ps[:], ident_sb[:], vbs[:],
                                     start=False, stop=True)

                    # out = po / s + bias (eps keeps pad slots finite)
                    ssb = scpool.tile([128, 8], dt.float32, tag="ssb")
                    nc.vector.tensor_scalar_add(ssb[:], ps[:], 1e-30)
                    nc.vector.reciprocal(ssb[:], ssb[:])
                    osb = eopool.tile([128, 512], dt.float32)
                    o4 = osb[:].rearrange("p (h d) -> p h d", d=DH)
                    nc.vector.tensor_tensor(
                        o4, po[:].rearrange("p (h d) -> p h d", d=DH),
                        ssb[:].to_broadcast([128, 8, DH]),
                        op=mybir.AluOpType.mult)
                    oadd = nc.vector.tensor_tensor(
                        osb[:], osb[:], bias_sb[:], op=mybir.AluOpType.add)
                    last_vec_name = oadd.ins.name
                    nc.sync.dma_start(out_d[t * 128:(t + 1) * 128, :], osb[:])
            for p in reversed(p2ctx):
                p.__exit__(None, None, None)
    nc.compile()
    return nc


# ------------------------------------------------------------------- driver
def kernel(x_src, x_dst, edge_src, edge_dst, W, attn_l, attn_r, bias):
    shared, per_core, k_tile, nch = _host_prep(
        np.asarray(x_src), np.asarray(x_dst), np.asarray(edge_src),
        np.asarray(edge_dst), np.asarray(W), np.asarray(attn_l),
        np.asarray(attn_r), np.asarray(bias))

    nc = _build_nc(k_tile, nch)

    in_maps = []
    for c in range(NCORES):
        in_maps.append({"xT": per_core[c]["xT"], "Wext": shared["Wext"],
                        "bias_rep": shared["bias_rep"],
                        "ident": shared["ident"],
                        "selT": per_core[c]["selT"],
                        "selD": per_core[c]["selD"],
                        "zidx": per_core[c]["zidx"]})

    def unperm(out_core, c):
        full = np.zeros((DPC, 512), F32)
        perm = per_core[c]["perm"]
        valid = perm >= 0
        full[perm[valid]] = out_core[np.nonzero(valid)[0]]
        return full

    if os.environ.get("KERNEL_SIM"):
        from concourse.bass_interp import CoreSim
        sim = CoreSim(nc, trace=False)
        cid = int(os.environ.get("KERNEL_SIM_CORE", "0"))
        for name, arr in in_maps[cid].items():
            sim.tensor(name)[:] = arr
        sim.simulate()
        out = unperm(np.array(sim.tensor("out")), cid)
        return np.concatenate([out] * NCORES, 0)  # selected core's slice only

    from concourse.bass_utils import run_bass_kernel_spmd
    res = run_bass_kernel_spmd(nc, in_maps, core_ids=list(range(NCORES)),
                               trace=bool(os.environ.get("KERNEL_TRACE")))
    global LAST_RESULTS
    LAST_RESULTS = res
    return np.concatenate([unperm(r["out"], c)
                           for c, r in enumerate(res.results)], 0)


LAST_RESULTS = None
